# revision 14
# baseline (speedup 1.0000x reference)
"""Trainium2 Bass kernel for DGL-style temporal GAT (dense attention over W nodes).

Math (per batch b, head h):
  z = x @ Wfc                       [W, H*D]
  el[s] = z[s] . attn_l_h ; er[t] = z[t] . attn_r_h
  e[s,t] = leaky_relu(el[s] + er[t], 0.2)
  alpha = softmax_s(e) ; out[t] = sum_s alpha[s,t] z[s] + gat_bias
  y = out @ Wproj + bproj

Key identity used on-device:
  exp(leaky(v)) = e^{0.2 el} * max(e^{0.8 el} e^{0.8 er}, 1) * e^{0.2 er}
The e^{0.2 er} column factor cancels in the softmax, so the [s,t] score tensor is
built with ONE fused DVE tensor_scalar op per [128,512] chunk:
  u[s,t] = max(Er8_bcast[t] * El1[s], El2[s]),  El1 = e^{el}, El2 = e^{0.2 el}, Er8 = e^{0.8 er}
Numerator+denominator come from one PE matmul with lhsT = [z_h | ones] (33 cols).

Raw Bass (explicit engine streams + semaphores); data-parallel over batch B=32
across 8 cores (4 batches per core).
"""

from contextlib import ExitStack

import numpy as np

import concourse.bass as bass
from concourse import mybir
from concourse.bass_utils import run_bass_kernel_spmd

F32 = mybir.dt.float32
AF = mybir.ActivationFunctionType
OP = mybir.AluOpType

B, W, F = 32, 512, 256
H, D = 8, 32
HD = H * D
NCORES = 8
BL = B // NCORES            # batches per core
TOK = BL * W                # tokens per core
P = 128
NT = TOK // P               # token tiles per core
KC = F // P                 # contraction chunks over F
TB = W // P                 # s-chunks per batch
ZW = D + 1                  # 33: z cols + ones col per head
ZAUGW = H * ZW              # 264
NK = BL * H                 # 32 (b,h) attention blocks per core
NJ = NT                     # output tiles


def _bcast_ap(row_ap, nparts):
    return bass.AP(
        tensor=row_ap.tensor,
        offset=row_ap.offset,
        ap=[[0, nparts]] + [list(d) for d in row_ap.ap[1:]],
    )


def build_program():
    nc = bass.Bass()
    x_t = nc.declare_dram_parameter("x_t", [F, TOK], F32, isOutput=False)
    wfc_aug = nc.declare_dram_parameter("wfc_aug", [F, ZAUGW + H], F32, isOutput=False)
    wfc_ar = nc.declare_dram_parameter("wfc_ar", [F, H], F32, isOutput=False)
    wproj = nc.declare_dram_parameter("wproj", [HD, F], F32, isOutput=False)
    bp = nc.declare_dram_parameter("bp", [1, F], F32, isOutput=False)
    y = nc.declare_dram_parameter("y", [TOK, F], F32, isOutput=True)

    ZCOLS = ZAUGW + H  # 272

    # ---------------- static schedules & tick tables ----------------
    # ACT op-group order (1 tick per group)
    act_ops = [("zgrp", i) for i in range(NT)] + [("er8", j) for j in range(TB)]
    for k in range(NK):
        act_ops.append(("erbev", k))
        if k >= 1:
            act_ops.append(("rbev", k - 1))
        b_, h_ = divmod(k, H)
        if h_ == 2 and b_ >= 1:
            act_ops += [("pspev", b_ - 1, j) for j in range(TB)]
    act_ops.append(("rbev", NK - 1))
    act_ops += [("pspev", BL - 1, j) for j in range(TB)]
    act_tick = {op: t + 1 for t, op in enumerate(act_ops)}

    # PE op order (1 tick per group)
    pe_ops = [("z", i) for i in range(NT)] + [("er", j) for j in range(TB)]
    pe_ops += [("erb", 0), ("erb", 1)]
    for k in range(NK):
        pe_ops.append(("apply", k))
        if k >= 1:
            pe_ops.append(("rbmm", k - 1))
        if k + 2 < NK:
            pe_ops.append(("erb", k + 2))
        b_, h_ = divmod(k, H)
        if h_ == 2 and b_ >= 1:
            pe_ops += [("proj", b_ - 1, j) for j in range(TB)]
    pe_ops.append(("rbmm", NK - 1))
    pe_ops += [("proj", BL - 1, j) for j in range(TB)]
    pe_tick = {op: t + 1 for t, op in enumerate(pe_ops)}

    # DVE op order (1 tick per op)
    dve_ops = [("ones",)] + [("zones", i) for i in range(NT)]
    for k in range(NK):
        dve_ops += [("u", k, c) for c in range(TB)]
        if k >= 1:
            dve_ops.append(("recip", k - 1))
        if k >= 2:
            dve_ops.append(("norm", k - 2))
    dve_ops += [("recip", NK - 1), ("norm", NK - 2), ("norm", NK - 1)]
    dve_tick = {op: t + 1 for t, op in enumerate(dve_ops)}


    with ExitStack() as es:
        def sbuf(name, shape):
            return es.enter_context(nc.sbuf_tensor(name, shape, F32))

        def psum(name, shape):
            return es.enter_context(nc.psum_tensor(name, shape, F32))

        x_sb = sbuf("x_sb", [P, KC, TOK])
        wfc_sb = sbuf("wfc_sb", [P, KC, ZCOLS])
        war_sb = sbuf("war_sb", [P, KC, H])
        wpr_sb = sbuf("wpr_sb", [P, KC, F])
        bp_sb = sbuf("bp_sb", [1, F])
        ones_sb = sbuf("ones_sb", [1, P])
        zaug = sbuf("zaug", [P, NT, ZAUGW])
        el1 = sbuf("el1", [P, NT, H])
        el2 = sbuf("el2", [P, NT, H])
        er8 = sbuf("er8", [H, TOK])
        er8_flat = sbuf("er8_flat", [1, H * TOK])
        erb_sb = sbuf("erb_sb", [P, 3, W])
        u_sb = sbuf("u_sb", [P, 8, W])
        r_sb = sbuf("r_sb", [1, 3, W])
        rb_sb = sbuf("rb_sb", [D, 3, W])
        zo_sb = sbuf("zo_sb", [P, 2, 2, W])     # [part, bset, hd-block, t]
        out_sb = sbuf("out_sb", [P, 3, F])

        ps_z = [psum("ps_z0", [P, W]), psum("ps_z1", [P, W])]   # z-pass / erb bcast
        ps_er = psum("ps_er", [D, W])
        ps_o = [psum(f"ps_o{i}", [ZW, W]) for i in range(3)]
        ps_p = [psum("ps_p0", [P, F]), psum("ps_p1", [P, F])]

        s_w = es.enter_context(nc.semaphore("s_w"))
        s_x = [es.enter_context(nc.semaphore(f"s_x{j}")) for j in range(TB)]
        s_ef = [es.enter_context(nc.semaphore(f"s_ef{j}")) for j in range(TB)]
        s_pe = es.enter_context(nc.semaphore("s_pe"))
        s_act = es.enter_context(nc.semaphore("s_act"))
        s_dve = es.enter_context(nc.semaphore("s_dve"))
        s_y = es.enter_context(nc.semaphore("s_y"))

        with nc.Block() as block:

            @block.sync
            def _(eng):
                # input DMAs: weights wave then x segments
                for c in range(KC):
                    eng.dma_start(out=wfc_sb[:, c, :],
                                  in_=wfc_aug[c * P:(c + 1) * P, :]).then_inc(s_w, 16)
                for c in range(KC):
                    eng.dma_start(out=war_sb[:, c, :],
                                  in_=wfc_ar[c * P:(c + 1) * P, :]).then_inc(s_w, 16)
                for c in range(KC):
                    eng.dma_start(out=wpr_sb[:, c, :],
                                  in_=wproj[c * P:(c + 1) * P, :]).then_inc(s_w, 16)
                eng.dma_start(out=bp_sb[:, :], in_=bp[:, :]).then_inc(s_w, 16)
                for j in range(TB):
                    for c in range(KC):
                        eng.dma_start(
                            out=x_sb[:, c, j * W:(j + 1) * W],
                            in_=x_t[c * P:(c + 1) * P, j * W:(j + 1) * W],
                        ).then_inc(s_x[j], 16)
                # er8 rows -> partition-0 flat copy (matmul rhs needs base partition 0)
                for j in range(TB):
                    eng.wait_ge(s_act, act_tick[("er8", j)])
                    seg = er8[:, j * W:(j + 1) * W]
                    base = er8_flat[0:1, :]
                    dst = bass.AP(
                        tensor=base.tensor,
                        offset=j * W,
                        ap=[list(base.ap[0]), [TOK, H], [1, W]],
                    )
                    eng.dma_start(out=dst, in_=seg).then_inc(s_ef[j], 16)

            @block.tensor
            def _(eng):
                for op in pe_ops:
                    if op[0] == "z":
                        i = op[1]
                        if i == 0:
                            eng.wait_ge(s_w, 112)
                        eng.wait_ge(s_x[i // TB], 32)
                        if i >= 2:
                            eng.wait_ge(s_act, act_tick[("zgrp", i - 2)])
                        pz = ps_z[i % 2]
                        for c in range(KC):
                            mm = nc.tensor.matmul(
                                pz[:, 0:ZCOLS],
                                lhsT=x_sb[:, c, i * P:(i + 1) * P],
                                rhs=wfc_sb[:, c, :],
                                start=(c == 0), stop=(c == KC - 1),
                            )
                        mm.then_inc(s_pe, 1)
                    elif op[0] == "er":
                        j = op[1]
                        if j >= 1:
                            eng.wait_ge(s_act, act_tick[("er8", j - 1)])
                        for c in range(KC):
                            mm = nc.tensor.matmul(
                                ps_er[0:H, :],
                                lhsT=war_sb[:, c, :],
                                rhs=x_sb[:, c, j * W:(j + 1) * W],
                                start=(c == 0), stop=(c == KC - 1),
                            )
                        mm.then_inc(s_pe, 1)
                    elif op[0] == "erb":
                        k = op[1]
                        b_, h_ = divmod(k, H)
                        eng.wait_ge(s_ef[b_], 16)
                        eng.wait_ge(s_dve, dve_tick[("ones",)])
                        if k >= 2:
                            eng.wait_ge(s_act, act_tick[("erbev", k - 2)])
                        mm = nc.tensor.matmul(
                            ps_z[k % 2][:, :],
                            lhsT=ones_sb[:, :],
                            rhs=er8_flat[0:1, h_ * TOK + b_ * W:h_ * TOK + (b_ + 1) * W],
                            start=True, stop=True,
                        )
                        mm.then_inc(s_pe, 1)
                    elif op[0] == "rbmm":
                        k = op[1]
                        eng.wait_ge(s_dve, dve_tick[("recip", k)])
                        if k >= 1:
                            eng.wait_ge(s_act, act_tick[("rbev", k - 1)])
                        mm = nc.tensor.matmul(
                            ps_er[:, :],
                            lhsT=ones_sb[:, 0:D],
                            rhs=r_sb[:, k % 3, :],
                            start=True, stop=True,
                        )
                        mm.then_inc(s_pe, 1)
                    elif op[0] == "apply":
                        k = op[1]
                        b_, h_ = divmod(k, H)
                        if k >= 3:
                            eng.wait_ge(s_dve, dve_tick[("norm", k - 3)])
                        po = ps_o[k % 3]
                        for c in range(TB):
                            it = b_ * TB + c
                            eng.wait_ge(s_dve, dve_tick[("u", k, c)])
                            mm = nc.tensor.matmul(
                                po[:, :],
                                lhsT=zaug[:, it, h_ * ZW:(h_ + 1) * ZW],
                                rhs=u_sb[:, (TB * k + c) % 8, :],
                                start=(c == 0), stop=(c == TB - 1),
                            )
                        mm.then_inc(s_pe, 1)
                    elif op[0] == "proj":
                        b_, j = op[1], op[2]
                        if j == 0:
                            eng.wait_ge(s_dve, dve_tick[("norm", b_ * H + H - 1)])
                        prev = (b_ * TB + j) - 2
                        if prev >= 0:
                            eng.wait_ge(s_act, act_tick[("pspev", prev // TB, prev % TB)])
                        pp = ps_p[j % 2]
                        nc.tensor.matmul(pp[:, :], lhsT=zo_sb[:, b_ % 2, 0, j * P:(j + 1) * P],
                                         rhs=wpr_sb[:, 0, :], start=True, stop=False)
                        nc.tensor.matmul(pp[:, :], lhsT=zo_sb[:, b_ % 2, 1, j * P:(j + 1) * P],
                                         rhs=wpr_sb[:, 1, :], start=False, stop=False)
                        mm = nc.tensor.matmul(pp[:, :], lhsT=ones_sb[:, :], rhs=bp_sb[:, :],
                                              start=False, stop=True)
                        mm.then_inc(s_pe, 1)

            @block.scalar
            def _(eng):
                for op in act_ops:
                    if op[0] == "zgrp":
                        i = op[1]
                        eng.wait_ge(s_pe, pe_tick[("z", i)])
                        pz = ps_z[i % 2]
                        nc.scalar.copy(zaug[:, i, :], pz[:, 0:ZAUGW])
                        nc.scalar.activation(el1[:, i, :], pz[:, ZAUGW:ZCOLS], AF.Exp,
                                             scale=1.0)
                        a = nc.scalar.activation(el2[:, i, :], pz[:, ZAUGW:ZCOLS], AF.Exp,
                                                 scale=0.2)
                        a.then_inc(s_act, 1)
                    elif op[0] == "er8":
                        j = op[1]
                        eng.wait_ge(s_pe, pe_tick[("er", j)])
                        a = nc.scalar.activation(er8[:, j * W:(j + 1) * W], ps_er[0:H, :],
                                                 AF.Exp, scale=1.0)
                        a.then_inc(s_act, 1)
                    elif op[0] == "erbev":
                        k = op[1]
                        eng.wait_ge(s_pe, pe_tick[("erb", k)])
                        if k >= 3:
                            eng.wait_ge(s_dve, dve_tick[("u", k - 3, TB - 1)])
                        a = nc.scalar.copy(erb_sb[:, k % 3, :], ps_z[k % 2][:, :])
                        a.then_inc(s_act, 1)
                    elif op[0] == "rbev":
                        k = op[1]
                        eng.wait_ge(s_pe, pe_tick[("rbmm", k)])
                        if k >= 3:
                            eng.wait_ge(s_dve, dve_tick[("norm", k - 3)])
                        a = nc.scalar.copy(rb_sb[:, k % 3, :], ps_er[:, :])
                        a.then_inc(s_act, 1)
                    elif op[0] == "pspev":
                        b_, j = op[1], op[2]
                        eng.wait_ge(s_pe, pe_tick[("proj", b_, j)])
                        n = b_ * TB + j
                        if n >= 3:
                            eng.wait_ge(s_y, (n - 2) * 16)
                        a = nc.scalar.copy(out_sb[:, n % 3, :], ps_p[j % 2][:, :])
                        a.then_inc(s_act, 1)

            @block.vector
            def _(eng):
                for op in dve_ops:
                    if op[0] == "ones":
                        nc.vector.memset(ones_sb[:, :], 1.0).then_inc(s_dve, 1)
                    elif op[0] == "zones":
                        i = op[1]
                        eng.wait_ge(s_act, act_tick[("zgrp", i)])
                        zt = zaug[:, i, :].rearrange("p (h e) -> p h e", e=ZW)
                        nc.vector.memset(zt[:, :, D:ZW], 1.0).then_inc(s_dve, 1)
                    elif op[0] == "u":
                        k, c = op[1], op[2]
                        b_, h_ = divmod(k, H)
                        it = b_ * TB + c
                        eng.wait_ge(s_act, act_tick[("erbev", k)])
                        if k >= 2:
                            eng.wait_ge(s_pe, pe_tick[("apply", k - 2)])
                        nc.vector.tensor_scalar(
                            u_sb[:, (TB * k + c) % 8, :],
                            in0=erb_sb[:, k % 3, :],
                            scalar1=el1[:, it, h_:h_ + 1],
                            scalar2=el2[:, it, h_:h_ + 1],
                            op0=OP.mult,
                            op1=OP.max,
                        ).then_inc(s_dve, 1)
                    elif op[0] == "recip":
                        k = op[1]
                        eng.wait_ge(s_pe, pe_tick[("apply", k)])
                        if k >= 3:
                            eng.wait_ge(s_pe, pe_tick[("rbmm", k - 3)])
                        nc.vector.reciprocal(
                            r_sb[:, k % 3, :], ps_o[k % 3][D:ZW, :]
                        ).then_inc(s_dve, 1)
                    elif op[0] == "norm":
                        k = op[1]
                        b_, h_ = divmod(k, H)
                        eng.wait_ge(s_act, act_tick[("rbev", k)])
                        if b_ >= 2 and h_ == 0:
                            eng.wait_ge(s_pe, pe_tick[("proj", b_ - 2, TB - 1)])
                        blk, row = h_ // 4, (h_ % 4) * D
                        nc.vector.tensor_mul(
                            zo_sb[row:row + D, b_ % 2, blk, :],
                            ps_o[k % 3][0:D, :],
                            rb_sb[:, k % 3, :],
                        ).then_inc(s_dve, 1)

            @block.gpsimd
            def _(eng):
                for b_ in range(BL):
                    for j in range(TB):
                        n = b_ * TB + j
                        eng.wait_ge(s_act, act_tick[("pspev", b_, j)])
                        if n >= 1:
                            eng.wait_ge(s_y, n * 16)
                        eng.dma_start(out=y[n * P:(n + 1) * P, :],
                                      in_=out_sb[:, n % 3, :]).then_inc(s_y, 16)
                eng.wait_ge(s_y, NT * 16)

    return nc


_NC = None


def _get_nc():
    global _NC
    if _NC is None:
        _NC = build_program()
    return _NC


def make_host_inputs(x, Wfc, attn_l, attn_r, gat_bias, Wproj, bproj):
    """Precompute folded weights + per-core sharded inputs (numpy)."""
    x = np.ascontiguousarray(np.asarray(x, np.float32))
    Wfc = np.asarray(Wfc, np.float32)
    attn_l = np.asarray(attn_l, np.float32)
    attn_r = np.asarray(attn_r, np.float32)
    gat_bias = np.asarray(gat_bias, np.float32)
    Wproj = np.asarray(Wproj, np.float32)
    bproj = np.asarray(bproj, np.float32)

    A_l = np.zeros((HD, H), np.float32)
    A_l[np.arange(HD), np.arange(HD) // D] = attn_l.reshape(-1)
    A_r = np.zeros((HD, H), np.float32)
    A_r[np.arange(HD), np.arange(HD) // D] = attn_r.reshape(-1)

    wfc_aug = np.zeros((F, ZAUGW + H), np.float32)
    wfc_aug[:, :ZAUGW].reshape(F, H, ZW)[:, :, :D] = Wfc.reshape(F, H, D)
    wfc_aug[:, ZAUGW:] = Wfc @ A_l                      # el columns (unscaled)
    wfc_ar = np.ascontiguousarray(0.8 * (Wfc @ A_r))    # er columns (0.8-scaled)
    bp = (gat_bias @ Wproj + bproj).reshape(1, F).astype(np.float32)

    shared = {
        "wfc_aug": wfc_aug,
        "wfc_ar": wfc_ar,
        "wproj": np.ascontiguousarray(Wproj),
        "bp": np.ascontiguousarray(bp),
    }
    in_maps = []
    for c in range(NCORES):
        x_c = x[c * BL:(c + 1) * BL].reshape(TOK, F)
        in_maps.append({"x_t": np.ascontiguousarray(x_c.T), **shared})
    return in_maps


def kernel(x, Wfc, attn_l, attn_r, gat_bias, Wproj, bproj):
    nc = _get_nc()
    in_maps = make_host_inputs(x, Wfc, attn_l, attn_r, gat_bias, Wproj, bproj)
    res = run_bass_kernel_spmd(nc, in_maps, list(range(NCORES)))
    out = np.empty((B, W, F), np.float32)
    for c in range(NCORES):
        out[c * BL:(c + 1) * BL] = res.results[c]["y"].reshape(BL, W, F)
    return out
